# revision 8
# baseline (speedup 1.0000x reference)
"""Multi-head attention (B=4, N=2048, E=512, H=8) on 8 TRN2 NeuronCores.

Sharding: 8-way data parallel over (batch, query-half). Each core computes,
for one batch b and one half of the query tokens (1024 of 2048):
  - full qkv projection (q only for its 1024 query tokens; k,v for all 2048
    keys, which costs a duplicated k/v projection but removes every
    cross-core dependency -- no collectives, no partial-sum all-reduce),
  - attention for all 8 heads over its query range,
  - the full output projection for its 1024 tokens.
The host then concatenates the 8 disjoint [1024, 512] output shards.

Key layout trick: activations are kept feature-major ("transposed") end to
end -- xT [E, N], qT/kT [feat, tok], scores as [keys, q] -- so the softmax
matmuls never need an on-chip transpose. The softmax denominator is obtained
for free by augmenting V with a ones-column (PV matmul computes both
attn_out.T and the row-sum), and the final 1/rsum normalization is applied
via a K=1 broadcast matmul + elementwise multiply.

Token permutation note: cores covering the second query half receive x with
the two token halves swapped (queries always = columns 0..1023 of xT).
Softmax and the PV contraction are permutation-invariant over keys, so the
result is unchanged.

All matmul operands are bitcast to float32r: full-rate (1 cycle/row) fp32
matmuls on the PE array vs 4 cycles/row for strict fp32.
"""

import numpy as np

import concourse.bass as bass
import concourse.mybir as mybir
import concourse.tile as tile
from concourse import bacc
from concourse.bass_utils import run_bass_kernel_spmd

B, N, E, H, D = 4, 2048, 512, 8, 64
NQ = N // 2          # query tokens per core
NCORES = 8
SCALE = D ** -0.5
F32 = mybir.dt.float32
F32R = mybir.dt.float32r
EXP = mybir.ActivationFunctionType.Exp

# Set by test harness to request an NTFF trace / exec time.
TRACE = False
LAST_RESULTS = None
DEBUG_TAPS = False


def _r(ap):
    if ap.dtype != F32R:
        return ap.bitcast(F32R)
    return ap


def build_program():
    nc = bacc.Bacc("TRN2", target_bir_lowering=False, debug=False,
                   num_devices=NCORES)

    xt_d = nc.dram_tensor("xt", [E, N], F32R, kind="ExternalInput")
    wqkvt_d = nc.dram_tensor("wqkvt", [E, 3 * E], F32R, kind="ExternalInput")
    # wpk[t][d, 512*s + e] = w_proj[e, 64*(2t+s) + d]  (head-pair packed)
    wpk_d = nc.dram_tensor("wpk", [4, D, 2 * E], F32R, kind="ExternalInput")
    bproj_d = nc.dram_tensor("bproj", [1, E], F32R, kind="ExternalInput")
    ones_d = nc.dram_tensor("ones", [1, 128], F32R, kind="ExternalInput")
    onesf_d = nc.dram_tensor("onesf", [1, 128], F32, kind="ExternalInput")
    vones_d = nc.dram_tensor("vones", [128, 8], F32R, kind="ExternalInput")
    y_d = nc.dram_tensor("y", [NQ, E], F32, kind="ExternalOutput")
    if DEBUG_TAPS:
        dbg_rsum_d = nc.dram_tensor("dbg_rsum", [16, 512], F32,
                                    kind="ExternalOutput")
        dbg_rr_d = nc.dram_tensor("dbg_rr", [16, 512], F32,
                                  kind="ExternalOutput")
        dbg_outT_d = nc.dram_tensor("dbg_outT", [4, D, 2 * NQ], F32R,
                                    kind="ExternalOutput")
        dbg_qt_d = nc.dram_tensor("dbg_qt", [128, NQ], F32R,
                                  kind="ExternalOutput")
        dbg_kt_d = nc.dram_tensor("dbg_kt", [128, N], F32R,
                                  kind="ExternalOutput")
        dbg_va_d = nc.dram_tensor("dbg_va", [128, 8 * (D + 1)], F32R,
                                  kind="ExternalOutput")
        dbg_ex_d = nc.dram_tensor("dbg_ex", [128, 1024], F32,
                                  kind="ExternalOutput")
        dbg_pv_d = nc.dram_tensor("dbg_pv", [D + 1, 512], F32,
                                  kind="ExternalOutput")

    with tile.TileContext(nc) as tc:
        with (
            # kt tiles + outT tiles share 5 rotating 8KB slots (kt[t] dies
            # right when outT[t] is born, at the end of pair t).
            tc.tile_pool(name="big", bufs=5) as pbig,
            tc.tile_pool(name="pers", bufs=1) as pers,
            tc.tile_pool(name="exp", bufs=2) as pexp,
            tc.tile_pool(name="stage", bufs=2) as pstage,
            tc.tile_pool(name="rrow", bufs=2) as prrow,
            tc.tile_pool(name="ysb", bufs=2) as pysb,
            tc.tile_pool(name="psS", bufs=2, space="PSUM") as psS,
            tc.tile_pool(name="psQ", bufs=2, space="PSUM") as psQ,
            tc.tile_pool(name="psP", bufs=2, space="PSUM") as psP,
        ):
            # ---------- persistent SBUF tensors ----------
            xt = [pers.tile([128, N], F32R, name=f"xt{i}", tag=f"xt{i}")
                  for i in range(4)]
            wq = [pers.tile([128, 3 * E], F32R, name=f"wq{i}", tag=f"wq{i}")
                  for i in range(4)]
            wpk = [pers.tile([D, 2 * E], F32R, name=f"wpk{i}", tag=f"wpk{i}")
                   for i in range(4)]
            qt = [pers.tile([128, NQ], F32R, name=f"qt{i}", tag=f"qt{i}")
                  for i in range(4)]
            vaug = [pers.tile([128, 8 * (D + 1)], F32R, name=f"va{j}",
                              tag=f"va{j}") for j in range(16)]
            bproj = pers.tile([1, E], F32R, name="bproj", tag="bproj")
            ones = pers.tile([1, 128], F32R, name="ones", tag="ones")
            onesf = pers.tile([1, 128], F32, name="onesf", tag="onesf")

            # kt tiles allocated up-front so outT allocations (later) chain
            # onto kt releases, not onto each other.
            kt = [pbig.tile([128, N], F32R, name=f"kt{t}", tag="big")
                  for t in range(4)]
            outT = [None] * 4  # [64, 2048] tiles, allocated at pair tails

            # ---------- input DMAs ----------
            for i in range(4):
                nc.sync.dma_start(xt[i][:], xt_d[128 * i:128 * (i + 1), :])
            for i in range(4):
                nc.sync.dma_start(wq[i][:], wqkvt_d[128 * i:128 * (i + 1), :])
            for t in range(4):
                nc.sync.dma_start(wpk[t][:], wpk_d[t])
            nc.sync.dma_start(bproj[:], bproj_d[:])
            nc.sync.dma_start(ones[:], ones_d[:])
            nc.sync.dma_start(onesf[:], onesf_d[:])

            # ---------- qkv projection ----------
            # qT tiles: q features 128m..128m+127 for my NQ query tokens.
            def emit_qt(m):
                for c in range(NQ // 512):
                    ps = psQ.tile([128, 512], F32, name=f"qtp{m}_{c}",
                                  tag="q")
                    for k in range(4):
                        nc.tensor.matmul(
                            ps[:],
                            _r(wq[k][:, 128 * m:128 * (m + 1)]),
                            _r(xt[k][:, 512 * c:512 * (c + 1)]),
                            start=(k == 0), stop=(k == 3))
                    nc.vector.tensor_copy(
                        qt[m][:, 512 * c:512 * (c + 1)], ps[:])

            # kT tiles: k features (offset E in wqkvt) for all N keys.
            def emit_kt(m):
                for c in range(N // 512):
                    ps = psQ.tile([128, 512], F32, name=f"ktp{m}_{c}",
                                  tag="q")
                    for k in range(4):
                        nc.tensor.matmul(
                            ps[:],
                            _r(wq[k][:, E + 128 * m:E + 128 * (m + 1)]),
                            _r(xt[k][:, 512 * c:512 * (c + 1)]),
                            start=(k == 0), stop=(k == 3))
                    nc.vector.tensor_copy(
                        kt[m][:, 512 * c:512 * (c + 1)], ps[:])

            # v in natural layout [tokens, feats], augmented with a ones
            # column per head: vaug[j][p, 65h + d] = v[128j + p, 64h + d],
            # vaug[j][p, 65h + 64] = 1.0
            def emit_v(j):
                ps = psQ.tile([128, 512], F32, name=f"vp{j}", tag="q")
                for k in range(4):
                    nc.tensor.matmul(
                        ps[:],
                        _r(xt[k][:, 128 * j:128 * (j + 1)]),
                        _r(wq[k][:, 2 * E:3 * E]),
                        start=(k == 0), stop=(k == 3))
                va = vaug[j].rearrange("p (h c) -> p h c", c=D + 1)
                nc.sync.dma_start(va[:, :, D], vones_d[:])
                nc.vector.tensor_copy(
                    va[:, :, 0:D],
                    ps[:].rearrange("p (h c) -> p h c", c=D))

            emit_qt(0)
            emit_kt(0)
            for j in range(16):
                emit_v(j)

            # ---------- attention ----------
            def emit_pair(t):
                hA, hB = 2 * t, 2 * t + 1
                for s in range(2):          # query 512-col halves
                    pvA = psP.tile([D + 1, 512], F32, name=f"pvA{t}_{s}",
                                   tag="p")
                    pvB = psP.tile([D + 1, 512], F32, name=f"pvB{t}_{s}",
                                   tag="p")
                    for j in range(16):
                        sc = psS.tile([128, 1024], F32, name=f"sc{t}{s}{j}",
                                      tag="s")
                        # scoresT[j-block, q] for both heads; B runs in the
                        # 64..127 row-group concurrently with A.
                        nc.tensor.matmul(
                            sc[:, 0:512],
                            _r(kt[t][0:64, 128 * j:128 * (j + 1)]),
                            _r(qt[t][0:64, 512 * s:512 * (s + 1)]),
                            start=True, stop=True)
                        nc.tensor.matmul(
                            sc[:, 512:1024],
                            _r(kt[t][64:128, 128 * j:128 * (j + 1)]),
                            _r(qt[t][64:128, 512 * s:512 * (s + 1)]),
                            start=True, stop=True)
                        ex = pexp.tile([128, 1024], F32R, name=f"ex{t}{s}{j}",
                                       tag="ex")
                        nc.scalar.activation(ex[:], sc[:], EXP, scale=SCALE)
                        if DEBUG_TAPS and t == 0 and s == 0 and j == 0:
                            exf = pexp.tile([128, 1024], F32, name="exf",
                                            tag="exf")
                            nc.vector.tensor_copy(exf[:], ex[:])
                            nc.sync.dma_start(dbg_ex_d[:], exf[:])
                        nc.tensor.matmul(
                            pvA[:], _r(vaug[j][:, 65 * hA:65 * hA + 65]),
                            _r(ex[:, 0:512]),
                            start=(j == 0), stop=(j == 15))
                        nc.tensor.matmul(
                            pvB[:], _r(vaug[j][:, 65 * hB:65 * hB + 65]),
                            _r(ex[:, 512:1024]),
                            start=(j == 0), stop=(j == 15))

                    if DEBUG_TAPS and t == 0 and s == 0:
                        pvf = pstage.tile([D + 1, 512], F32, name="pvf",
                                          tag="pvf")
                        nc.vector.tensor_copy(pvf[:], pvA[:])
                        nc.sync.dma_start(dbg_pv_d[:], pvf[:])
                    if outT[t] is None:
                        outT[t] = pbig.tile([D, 2 * NQ], F32R,
                                            name=f"outT{t}", tag="big")
                    # unnormalized attnT output [d, q] into outT; the rsum
                    # row (partition 64) is reciprocal'd in place on DVE,
                    # moved to partition 0 by a tiny DMA, then broadcast
                    # across 64 partitions with a K=1 matmul.
                    for h, pv in ((hA, pvA), (hB, pvB)):
                        blk = NQ * (h - 2 * t) + 512 * s
                        nc.vector.tensor_copy(
                            outT[t][:, blk:blk + 512], pv[0:D, :])
                        # rsum row: psum -> sbuf copy (lane-aligned at
                        # partition 64), DMA down to partition 0, then
                        # SBUF-only reciprocal on DVE.
                        st = pstage.tile([D + 1, 512], F32,
                                         name=f"st{h}_{s}", tag="st")
                        nc.vector.tensor_copy(st[D:D + 1, :], pv[D:D + 1, :])
                        rs0 = prrow.tile([1, 512], F32, name=f"rs{h}_{s}",
                                         tag="rs")
                        nc.sync.dma_start(rs0[:], st[D:D + 1, :])
                        if DEBUG_TAPS:
                            nc.sync.dma_start(
                                dbg_rsum_d[2 * h + s:2 * h + s + 1, :],
                                rs0[:])
                        rr = prrow.tile([1, 512], F32, name=f"rr{h}_{s}",
                                        tag="rr")
                        nc.vector.reciprocal_approx_fast(
                            out=rr[:], in_=rs0[:])
                        if DEBUG_TAPS:
                            nc.sync.dma_start(
                                dbg_rr_d[2 * h + s:2 * h + s + 1, :], rr[:])
                        bc = psP.tile([D, 512], F32, name=f"bc{h}_{s}",
                                      tag="p")
                        nc.tensor.matmul(bc[:], onesf[:, 0:D], rr[:],
                                         start=True, stop=True)
                        nc.vector.tensor_mul(outT[t][:, blk:blk + 512],
                                             outT[t][:, blk:blk + 512],
                                             bc[:])

            emit_pair(0)
            emit_qt(1)
            emit_kt(1)
            emit_pair(1)
            emit_qt(2)
            emit_kt(2)
            emit_pair(2)
            emit_qt(3)
            emit_kt(3)
            emit_pair(3)

            if DEBUG_TAPS:
                for t in range(4):
                    nc.sync.dma_start(dbg_outT_d[t], outT[t][:])
                nc.sync.dma_start(dbg_qt_d[:], qt[0][:])
                nc.sync.dma_start(dbg_kt_d[:], kt[0][:])
                nc.sync.dma_start(dbg_va_d[:], vaug[0][:])

            # ---------- output projection ----------
            # y[tok, e] = b_proj[e] + sum_h outT[h//2][d, tokblk] * wpk
            for tc_i in range(NQ // 128):
                yps = psQ.tile([128, 512], F32, name=f"yps{tc_i}", tag="q")
                nc.tensor.matmul(yps[:], _r(ones[:]), _r(bproj[:]),
                                 start=True, stop=False)
                for h in range(H):
                    t, s = h // 2, h % 2
                    nc.tensor.matmul(
                        yps[:],
                        _r(outT[t][:, NQ * s + 128 * tc_i:
                                   NQ * s + 128 * (tc_i + 1)]),
                        _r(wpk[t][:, 512 * s:512 * (s + 1)]),
                        start=False, stop=(h == H - 1))
                ysb = pysb.tile([128, 512], F32, name=f"ysb{tc_i}", tag="y")
                nc.vector.tensor_copy(ysb[:], yps[:])
                nc.sync.dma_start(y_d[128 * tc_i:128 * (tc_i + 1), :],
                                  ysb[:])

    nc.compile()
    return nc


_NC_CACHE = None


def _get_program():
    global _NC_CACHE
    if _NC_CACHE is None:
        _NC_CACHE = build_program()
    return _NC_CACHE


def make_in_maps(x, w_qkv, w_proj, b_proj):
    x = np.asarray(x, dtype=np.float32)
    w_qkv = np.asarray(w_qkv, dtype=np.float32)
    w_proj = np.asarray(w_proj, dtype=np.float32)
    b_proj = np.asarray(b_proj, dtype=np.float32)

    wqkvt = np.ascontiguousarray(w_qkv.T)                   # [512, 1536]
    # wpk[t][d, 512*s + e] = w_proj[e, 64*(2t+s) + d]
    wpk = np.empty((4, D, 2 * E), dtype=np.float32)
    for t in range(4):
        for s in range(2):
            h = 2 * t + s
            wpk[t][:, 512 * s:512 * (s + 1)] = w_proj[:, 64 * h:64 * h + 64].T
    bp = np.ascontiguousarray(b_proj.reshape(1, E))
    ones = np.ones((1, 128), dtype=np.float32)

    in_maps = []
    for c in range(NCORES):
        b, s = c // 2, c % 2
        xb = x[b]
        if s == 1:
            xb = np.concatenate([xb[NQ:], xb[:NQ]], axis=0)
        in_maps.append({
            "xt": np.ascontiguousarray(xb.T),
            "wqkvt": wqkvt,
            "wpk": wpk,
            "bproj": bp,
            "ones": ones,
            "onesf": ones,
            "vones": np.ones((128, 8), dtype=np.float32),
        })
    return in_maps


def kernel(x, w_qkv, w_proj, b_proj):
    global LAST_RESULTS
    nc = _get_program()
    in_maps = make_in_maps(x, w_qkv, w_proj, b_proj)
    res = run_bass_kernel_spmd(nc, in_maps, core_ids=list(range(NCORES)),
                               trace=TRACE)
    LAST_RESULTS = res
    y = np.empty((B, N, E), dtype=np.float32)
    for c in range(NCORES):
        b, s = c // 2, c % 2
        y[b, NQ * s:NQ * (s + 1), :] = res.results[c]["y"]
    return y
